# revision 13
# baseline (speedup 1.0000x reference)
"""CS-TreeLSTM (BRANCH=16, DEPTH=4, IN=HID=512) on 8 Trainium2 NeuronCores.

v2 strategy (data-parallel over subtrees):
  - Each core owns 8192 leaves + 512 level-3 nodes; host finishes levels
    2..0 in float64 (0.4% of FLOPs), and additionally precomputes exactly
    (f64) the level-3 x-part pre-activations (x3@W*x + b) and the forget
    x-term fx3 = x3@Wfx + bf, which the device consumes as PSUM pre-writes.
  - Leaf u,o gate matmuls run as fp8e4 DoubleRow (2 contraction rows per
    cycle) with a two-digit hi/lo decomposition on BOTH operands (3
    digit-product terms, lo*lo dropped).  Digits share one scale per side
    (x*16, W*64) so all terms accumulate in one PSUM group; the 1/1024
    descale rides the ACT evacuation's scale argument.  The i gate (most
    error-sensitive: its error multiplies u into C which the fcsum cascade
    amplifies ~10x) stays f32r, as do the Wfh@C forget matmuls and the
    level-3 h-part.
  - Elementwise: C-path (i,u,C) fp32; H-path (o,t,H) and f bf16; sums fp32.
  - Sibling sums are grouped free-dim tensor_reduce on DVE; f*C runs on
    the otherwise idle Pool engine, which also pre-writes the fx/l3-x
    terms into PSUM so those matmul groups accumulate on top (start=False).
  - Gate PSUM groups are 4-bank [128,4,512] tiles, two in flight
    (ping-pong) = all 8 banks.
"""

import sys

sys.path.insert(0, "/opt/trn_rl_repo")

import numpy as np

BRANCH = 16
DEPTH = 4
IN = 512
HID = 512
NC_N = 8
OFFS = [0, 1, 17, 273, 4369, 69905]
N_CHUNK = 16
LEAF_PER_CORE = 8192
L3_PER_CORE = 512

# which leaf gates use fp8 DoubleRow (others f32r)
FP8_GATES = {"i": False, "u": True, "o": True}
SX = 16.0  # x digit scale
SW = 64.0  # w digit scale

_CACHE = {}


def _build_nc():
    from concourse import bacc
    import concourse.mybir as mybir
    import concourse.tile as tile

    F32 = mybir.dt.float32
    F32R = mybir.dt.float32r
    BF16 = mybir.dt.bfloat16
    FP8 = mybir.dt.float8e4
    ACTF = mybir.ActivationFunctionType
    AX = mybir.AxisListType
    OP = mybir.AluOpType
    DR = mybir.MatmulPerfMode.DoubleRow

    any8 = any(FP8_GATES.values())
    anyr = not all(FP8_GATES.values())

    nc = bacc.Bacc()

    # ---- DRAM params ----
    x8 = nc.declare_dram_parameter("x8", [128, N_CHUNK, 4, 2, 512], FP8, isOutput=False) if any8 else None
    xr = nc.declare_dram_parameter("xr", [128, N_CHUNK, 4, 512], F32, isOutput=False) if anyr else None
    w8 = {}
    wr = {}
    for g in "iuo":
        if FP8_GATES[g]:
            w8[g] = nc.declare_dram_parameter("w8" + g, [128, 4, 2, 512], FP8, isOutput=False)
        else:
            wr[g] = nc.declare_dram_parameter("wr" + g, [128, 4, 512], F32, isOutput=False)
    wfh = nc.declare_dram_parameter("wfh", [128, 4, 512], F32, isOutput=False)
    fx3b = nc.declare_dram_parameter("fx3b", [128, 4, 512], F32, isOutput=False)
    l3p = {g: nc.declare_dram_parameter("l3p" + g, [128, 4, 512], F32, isOutput=False) for g in "iuo"}
    wh = {g: nc.declare_dram_parameter("wh" + g, [128, 4, 512], F32, isOutput=False) for g in "iuo"}
    bT = {g: nc.declare_dram_parameter("bT" + g, [128, 4], F32, isOutput=False) for g in "iuo"}
    out_t = {
        "i3T": nc.declare_dram_parameter("i3T", [128, 4, 512], BF16, isOutput=True),
        "u3T": nc.declare_dram_parameter("u3T", [128, 4, 512], BF16, isOutput=True),
        "o3T": nc.declare_dram_parameter("o3T", [128, 4, 512], BF16, isOutput=True),
        "fc3T": nc.declare_dram_parameter("fc3T", [128, 4, 512], F32, isOutput=True),
    }

    from contextlib import ExitStack

    with tile.TileContext(nc) as tc, ExitStack() as ctx:
        consts = ctx.enter_context(tc.tile_pool(name="consts", bufs=1))
        s8 = ctx.enter_context(tc.tile_pool(name="s8", bufs=2))
        sr = ctx.enter_context(tc.tile_pool(name="sr", bufs=2))
        gi = ctx.enter_context(tc.tile_pool(name="gi", bufs=2))
        gu = ctx.enter_context(tc.tile_pool(name="gu", bufs=2))
        go = ctx.enter_context(tc.tile_pool(name="go", bufs=2))
        gf = ctx.enter_context(tc.tile_pool(name="gf", bufs=1))
        gC = ctx.enter_context(tc.tile_pool(name="gC", bufs=3))
        gt = ctx.enter_context(tc.tile_pool(name="gt", bufs=1))
        gH = ctx.enter_context(tc.tile_pool(name="gH", bufs=1))
        gfc = ctx.enter_context(tc.tile_pool(name="gfc", bufs=1))
        longp = ctx.enter_context(tc.tile_pool(name="longp", bufs=1))
        psum = ctx.enter_context(tc.tile_pool(name="psum", bufs=4, space="PSUM"))

        # ---- constant loads (Pool SWDGE queue, parallel desc-gen) ----
        W8s, Wrs, bTs, L3p = {}, {}, {}, {}
        for g in "iuo":
            bTs[g] = consts.tile([128, 4], F32, tag="bT" + g, name="bT" + g)
            nc.gpsimd.dma_start(out=bTs[g][:, :], in_=bT[g][:, :])
        # ordered roughly by first use
        for g in "iuo" if not FP8_GATES["i"] else "uoi":
            if FP8_GATES[g]:
                W8s[g] = consts.tile([128, 4, 2, 512], FP8, tag="w8" + g, name="w8" + g)
                nc.gpsimd.dma_start(out=W8s[g][:, :, :, :], in_=w8[g][:, :, :, :])
            else:
                Wrs[g] = consts.tile([128, 4, 512], F32R, tag="wr" + g, name="wr" + g)
                nc.gpsimd.dma_start(out=Wrs[g][:, :, :], in_=wr[g][:, :, :].bitcast(F32R))
        Wfh = consts.tile([128, 4, 512], F32R, tag="wfh", name="wfh")
        Fx3b = consts.tile([128, 4, 512], F32, tag="fx3b", name="fx3b")
        for g in "iuo":
            L3p[g] = consts.tile([128, 4, 512], F32, tag="l3p" + g, name="l3p" + g)

        # persistent accumulators
        hsum3T = longp.tile([128, 4, 512], F32R, tag="hsum3T")
        fcsum3T = longp.tile([128, 4, 512], F32, tag="fcsum3T")

        def load_x8(c):
            t = s8.tile([128, 4, 2, 512], FP8, tag="x8", name=f"x8_{c}")
            nc.sync.dma_start(out=t[:, :, :, :], in_=x8[:, c, :, :, :])
            return t

        def load_xr(c):
            t = sr.tile([128, 4, 512], F32R, tag="xr", name=f"xr_{c}")
            nc.sync.dma_start(out=t[:, :, :], in_=xr[:, c, :, :].bitcast(F32R))
            return t

        def ps_pair():
            # two 2-bank half tiles per gate group; 4-deep ring = 8 banks,
            # so the PE can run ~2 gate-halves ahead of the ACT evacuations
            return [psum.tile([128, 2, 512], F32, tag="ps", name="ps") for _ in range(2)]

        def ps_m(pp, m):
            return pp[m // 2][:, m % 2, :]

        def mm_fp8(pp, Wt, xt):
            """3-digit-term fp8 DoubleRow product into half-tile pair.
            Wt [128,4k,2(hi,lo),512m]; xt [128,4k,2(lo,hi),512n]."""
            for m in range(4):
                ms = slice(m * 128, (m + 1) * 128)
                for kp in range(2):  # hi@hi, k-pair packed
                    nc.tensor.matmul(
                        ps_m(pp, m),
                        Wt[:, 2 * kp : 2 * kp + 2, 0, ms],
                        xt[:, 2 * kp : 2 * kp + 2, 1, :],
                        start=(kp == 0), stop=False, perf_mode=DR,
                    )
                for k in range(4):  # cross: (W_hi,W_lo) x (x_lo,x_hi)
                    nc.tensor.matmul(
                        ps_m(pp, m),
                        Wt[:, k, :, ms],
                        xt[:, k, :, :],
                        start=False, stop=(k == 3), perf_mode=DR,
                    )

        def mm_f32r(pp, Wt, xt):
            for m in range(4):
                ms = slice(m * 128, (m + 1) * 128)
                for k in range(4):
                    nc.tensor.matmul(
                        ps_m(pp, m), Wt[:, k, ms], xt[:, k, :],
                        start=(k == 0), stop=(k == 3),
                    )

        def gate_mm(g, x8t, xrt):
            pp = ps_pair()
            if FP8_GATES[g]:
                mm_fp8(pp, W8s[g], x8t)
            else:
                mm_f32r(pp, Wrs[g], xrt)
            return pp

        def gate_evac(g, pp, pool, dtype):
            sc = 1.0 / (SX * SW) if FP8_GATES[g] else 1.0
            sb = pool.tile([128, 4, 512], dtype, tag=g)
            act = ACTF.Tanh if g == "u" else ACTF.Sigmoid
            for m in range(4):  # bias varies per m-tile
                nc.scalar.activation(
                    sb[:, m, :], ps_m(pp, m), act,
                    bias=bTs[g][:, m : m + 1], scale=sc,
                )
            return sb

        def fpath_mm(C_prev):
            pp = ps_pair()
            mm_f32r(pp, Wfh, C_prev)
            return pp

        def fpath_add(c, pp):
            # DVE folds the broadcast fx3(+bias) term in, off the PE path
            f_sb = gf.tile([128, 4, 512], BF16, tag="f")
            for h in range(2):
                nc.vector.tensor_tensor(
                    out=f_sb[:, 2 * h : 2 * h + 2, :].rearrange("p t (g w) -> p t g w", w=16),
                    in0=pp[h][:, :, :].rearrange("p t (g w) -> p t g w", w=16),
                    in1=Fx3b[:, 2 * h : 2 * h + 2, 32 * c : 32 * c + 32][:, :, :, None]
                    .broadcast_to([128, 2, 32, 16]),
                    op=OP.add,
                )
            return f_sb

        def fpath_fc(c, f_sb, C_prev):
            fC_sb = gfc.tile([128, 4, 512], F32, tag="fC")
            nc.gpsimd.tensor_mul(fC_sb[:, :, :], f_sb[:, :, :], C_prev[:, :, :].bitcast(F32))
            nc.vector.tensor_reduce(
                fcsum3T[:, :, 32 * c : 32 * c + 32],
                fC_sb[:, :, :].rearrange("p t (g w) -> p t g w", w=16),
                axis=AX.X, op=OP.add,
            )

        # ---------------- leaf phase ----------------
        # per chunk c: PE runs f-mm(c-1), i-mm(c), u-mm(c), o-mm(c); the ACT
        # queue is ordered so no evacuation sits behind sigmoid-f / tanh-C.
        pipe = None
        prevHt = None
        Whs = {}
        for c in range(N_CHUNK):
            x8t = load_x8(c) if any8 else None
            xrt = load_xr(c) if anyr else None
            if c == 0:
                nc.gpsimd.dma_start(out=Wfh[:, :, :], in_=wfh[:, :, :].bitcast(F32R))
                nc.gpsimd.dma_start(out=Fx3b[:, :, :], in_=fx3b[:, :, :])
            if 2 <= c <= 4:
                g = "iuo"[c - 2]
                nc.gpsimd.dma_start(out=L3p[g][:, :, :], in_=l3p[g][:, :, :])
            if c >= 13:
                g = "iuo"[c - 13]
                Whs[g] = sr.tile([128, 4, 512], F32R, tag="wh" + g, bufs=1, name="wh" + g)
                nc.gpsimd.dma_start(out=Whs[g][:, :, :], in_=wh[g][:, :, :].bitcast(F32R))

            if prevHt is not None:
                # previous chunk's H product + sibling sum: both inputs are
                # ready at chunk start, so they lead the Pool/DVE queues
                po, pt, pc = prevHt
                H_sb = gH.tile([128, 4, 512], BF16, tag="H")
                nc.gpsimd.tensor_mul(H_sb[:, :, :], po[:, :, :], pt[:, :, :])
                with nc.allow_low_precision("f32r rounding for l3 h matmul"):
                    nc.vector.tensor_reduce(
                        hsum3T[:, :, 32 * pc : 32 * pc + 32],
                        H_sb[:, :, :].rearrange("p t (g w) -> p t g w", w=16),
                        axis=AX.X, op=OP.add,
                    )
            if pipe is not None:
                ppf = fpath_mm(pipe[1])
                f_sb = fpath_add(pipe[0], ppf)
            pp_i = gate_mm("i", x8t, xrt)
            i_sb = gate_evac("i", pp_i, gi, F32)
            if pipe is not None:
                nc.scalar.activation(f_sb[:, :, :], f_sb[:, :, :], ACTF.Sigmoid)
            pp_u = gate_mm("u", x8t, xrt)
            u_sb = gate_evac("u", pp_u, gu, F32)
            C_sb = gC.tile([128, 4, 512], F32R, tag="C")
            nc.vector.tensor_mul(C_sb[:, :, :], i_sb[:, :, :], u_sb[:, :, :])
            if pipe is not None:
                fpath_fc(pipe[0], f_sb, pipe[1])
            pp_o = gate_mm("o", x8t, xrt)
            t_sb = gt.tile([128, 4, 512], BF16, tag="t")
            nc.scalar.activation(t_sb[:, :, :], C_sb[:, :, :].bitcast(F32), ACTF.Tanh)
            o_sb = gate_evac("o", pp_o, go, BF16)
            prevHt = (o_sb, t_sb, c)
            pipe = (c, C_sb)

        # drain the pipeline: last f-path and last H/hsum
        ppf = fpath_mm(pipe[1])
        f_sb = fpath_add(pipe[0], ppf)
        nc.scalar.activation(f_sb[:, :, :], f_sb[:, :, :], ACTF.Sigmoid)
        fpath_fc(pipe[0], f_sb, pipe[1])
        po, pt, pc = prevHt
        H_sb = gH.tile([128, 4, 512], BF16, tag="H")
        nc.gpsimd.tensor_mul(H_sb[:, :, :], po[:, :, :], pt[:, :, :])
        with nc.allow_low_precision("f32r rounding for l3 h matmul"):
            nc.vector.tensor_reduce(
                hsum3T[:, :, 32 * pc : 32 * pc + 32],
                H_sb[:, :, :].rearrange("p t (g w) -> p t g w", w=16),
                axis=AX.X, op=OP.add,
            )

        # ---------------- level 3 ----------------
        nc.sync.dma_start(out=out_t["fc3T"][:, :, :], in_=fcsum3T[:, :, :])
        out_pool = {"i": gi, "u": gu, "o": go}
        for g in "iuo":
            pp = ps_pair()
            mm_f32r(pp, Whs[g], hsum3T)
            pre = gC.tile([128, 4, 512], F32, tag="C", name="l3pre" + g)
            for h in range(2):
                nc.vector.tensor_tensor(
                    out=pre[:, 2 * h : 2 * h + 2, :], in0=pp[h][:, :, :],
                    in1=L3p[g][:, 2 * h : 2 * h + 2, :], op=OP.add,
                )
            sb = out_pool[g].tile([128, 4, 512], BF16, tag=g)
            act = ACTF.Tanh if g == "u" else ACTF.Sigmoid
            nc.scalar.activation(sb[:, :, :], pre[:, :, :], act)
            nc.sync.dma_start(out=out_t[g + "3T"][:, :, :], in_=sb[:, :, :])

    nc.finalize()
    return nc


def _np_sigmoid(v):
    return 1.0 / (1.0 + np.exp(-v))


def _q8(a):
    import ml_dtypes

    return np.asarray(a, np.float32).astype(ml_dtypes.float8_e4m3)


def _host_prep(x, wi_w, wo_w, wu_w, wf_w, wi_b, wo_b, wu_b, wf_b):
    f8 = np.float64
    x = np.asarray(x, np.float32)
    Wg = {"i": np.asarray(wi_w), "o": np.asarray(wo_w), "u": np.asarray(wu_w)}
    Bg = {"i": np.asarray(wi_b), "o": np.asarray(wo_b), "u": np.asarray(wu_b)}
    wf = np.asarray(wf_w)
    bf = np.asarray(wf_b)

    def t_tiles(a2d):
        # [512, n] -> [128p, 4t, n] with row r = t*128 + p
        return np.ascontiguousarray(a2d.reshape(4, 128, a2d.shape[1]).transpose(1, 0, 2))

    common = {}
    for g in "iuo":
        wx = np.ascontiguousarray(Wg[g][:, :IN].T).astype(np.float32)  # [512in, 512hid]
        if FP8_GATES[g]:
            wh_ = _q8(wx * SW)
            wl_ = _q8(wx * SW - wh_.astype(np.float32))
            pair = np.stack([wh_, wl_], axis=1)  # [512k, 2(hi,lo), 512m]
            common["w8" + g] = np.ascontiguousarray(
                pair.reshape(4, 128, 2, 512).transpose(1, 0, 2, 3)
            )
        else:
            common["wr" + g] = t_tiles(wx)
        common["wh" + g] = t_tiles(np.ascontiguousarray(Wg[g][:, IN:].T).astype(np.float32))
        common["bT" + g] = np.ascontiguousarray(np.asarray(Bg[g]).reshape(4, 128).T)
    common["wfh"] = t_tiles(np.ascontiguousarray(wf[:, IN:].T).astype(np.float32))

    # exact (f64) host precompute: level-3 x-part pre-activations and fx3
    X3 = np.asarray(x[OFFS[3] : OFFS[4]], f8)  # [4096, 512]
    fx3 = (X3 @ np.asarray(wf[:, :IN], f8).T + np.asarray(bf, f8)).astype(np.float32)
    l3pre = {
        g: (X3 @ np.asarray(Wg[g][:, :IN], f8).T + np.asarray(Bg[g], f8)).astype(np.float32)
        for g in "iuo"
    }

    in_maps = []
    for core in range(NC_N):
        m = dict(common)
        xl = x[OFFS[4] + LEAF_PER_CORE * core : OFFS[4] + LEAF_PER_CORE * (core + 1)]
        xlT = np.ascontiguousarray(xl.T)  # [512, 8192]
        tiles = xlT.reshape(4, 128, N_CHUNK, 512)  # [k, p, c, n]
        if any(FP8_GATES.values()):
            xh_ = _q8(tiles * SX)
            xlo = _q8(tiles * SX - xh_.astype(np.float32))
            # pair order (lo, hi); layout [128p, 16c, 4k, 2, 512n]
            m["x8"] = np.ascontiguousarray(
                np.stack([xlo, xh_], axis=3).transpose(1, 2, 0, 3, 4)
            )
        if not all(FP8_GATES.values()):
            m["xr"] = np.ascontiguousarray(tiles.transpose(1, 2, 0, 3))
        sl3 = slice(L3_PER_CORE * core, L3_PER_CORE * (core + 1))
        m["fx3b"] = t_tiles(np.ascontiguousarray(fx3[sl3].T))
        for g in "iuo":
            m["l3p" + g] = t_tiles(np.ascontiguousarray(l3pre[g][sl3].T))
        in_maps.append(m)
    return in_maps


def _t_to_nodes(a):
    """[128, 4, n] transposed tile -> [n, 512] natural (hid = t*128 + p)."""
    a = np.asarray(a)
    return np.ascontiguousarray(np.transpose(a, (2, 1, 0)).reshape(a.shape[2], 512))


def _host_finish(x, res, wi_w, wi_b, wf_w, wf_b, wo_w, wo_b, wu_w, wu_b):
    f8 = np.float64
    i3 = np.concatenate([_t_to_nodes(res[c]["i3T"]) for c in range(NC_N)]).astype(f8)
    u3 = np.concatenate([_t_to_nodes(res[c]["u3T"]) for c in range(NC_N)]).astype(f8)
    o3 = np.concatenate([_t_to_nodes(res[c]["o3T"]) for c in range(NC_N)]).astype(f8)
    fc3 = np.concatenate([_t_to_nodes(res[c]["fc3T"]) for c in range(NC_N)]).astype(f8)

    C = i3 * u3 + fc3
    H = o3 * np.tanh(C)

    wi = np.asarray(wi_w, f8)
    wo = np.asarray(wo_w, f8)
    wu = np.asarray(wu_w, f8)
    wf = np.asarray(wf_w, f8)
    bi, bo, bu, bf = (np.asarray(b, f8) for b in (wi_b, wo_b, wu_b, wf_b))

    for d in range(2, -1, -1):
        Xd = np.asarray(x[OFFS[d] : OFFS[d + 1]], f8)
        n = Xd.shape[0]
        Hc = H.reshape(n, BRANCH, HID)
        Cc = C.reshape(n, BRANCH, HID)
        h_sum = Hc.sum(axis=1)
        xh = np.concatenate([Xd, h_sum], axis=1)
        i = _np_sigmoid(xh @ wi.T + bi)
        o = _np_sigmoid(xh @ wo.T + bo)
        u = np.tanh(xh @ wu.T + bu)
        fx = Xd @ wf[:, :IN].T
        fc = (C @ wf[:, IN:].T).reshape(n, BRANCH, HID)
        f = _np_sigmoid(fc + fx[:, None, :] + bf)
        C = i * u + (f * Cc).sum(axis=1)
        H = o * np.tanh(C)

    return H[0].astype(np.float32), C[0].astype(np.float32)


def _run(in_maps, trace=False):
    from concourse.bass_utils import run_bass_kernel_spmd

    if "nc" not in _CACHE:
        _CACHE["nc"] = _build_nc()
    return run_bass_kernel_spmd(_CACHE["nc"], in_maps, list(range(NC_N)), trace=trace)


def kernel(x, wi_w, wi_b, wf_w, wf_b, wo_w, wo_b, wu_w, wu_b, _trace=False):
    x = np.asarray(x, np.float32)
    in_maps = _host_prep(x, wi_w, wo_w, wu_w, wf_w, wi_b, wo_b, wu_b, wf_b)
    res = _run(in_maps, trace=_trace)
    _CACHE["last_results"] = res
    H0, C0 = _host_finish(x, res.results, wi_w, wi_b, wf_w, wf_b, wo_w, wo_b, wu_w, wu_b)
    return H0, C0


# revision 32
# speedup vs baseline: 1.5670x; 1.5670x over previous
"""CS-TreeLSTM (BRANCH=16, DEPTH=4, IN=HID=512) on 8 Trainium2 NeuronCores.

Strategy (data-parallel over subtrees, per the sharding hint):
  - Each core owns 8192 leaves and 512 level-3 nodes; levels 2..0 run on the
    host in float64 from the per-core level-3 outputs (i3/u3/o3/fcsum3),
    removing the serial small-matmul device tail entirely.
  - Activations live transposed on-chip: [hid/in on partitions, nodes on free].
  - Gate matmuls run as f32r (fp32-reduced, ~tf32) at bf16 PE speed.
  - Sibling sums (h_sum, sum_k f_k*C_k) are grouped free-dim reduces on DVE.
  - The parent-x + bias term of the forget gate is pre-written into PSUM by
    the (otherwise idle) Pool engine as a broadcast copy of fx3T; the Wfh
    matmuls then accumulate on top (start=False), replacing the indicator
    aug-matmul (which cost a full 512-row PE pass per m-tile).
  - Gates run i, u, o per chunk so C=i*u is ready while o still streams,
    hiding the f-path and hsum latency of the following consumers.
  - Input DMAs are spread across the SP, Pool and DVE queues: descriptor
    generation (~0.6us per DMA) serializes per queue and was the startup
    critical path.

Built on bacc.Bacc so multi-semaphore waits are legalized into event
semaphores automatically (TRN2 allows one sync wait per instruction).
"""

import sys

sys.path.insert(0, "/opt/trn_rl_repo")

import numpy as np

BRANCH = 16
DEPTH = 4
IN = 512
HID = 512
NC_N = 8
SIZES = [BRANCH**d for d in range(DEPTH + 1)]  # [1,16,256,4096,65536]
OFFS = [0, 1, 17, 273, 4369, 69905]
XT_COLS = 8192 + 512  # leaf x + level-3 x
C3_OFF = 8192
N_CHUNK = 16

_CACHE = {}


def _build_nc(cfg=None):
    cfg = cfg or {}
    from concourse import bacc
    import concourse.mybir as mybir
    import concourse.tile as tile

    F32 = mybir.dt.float32
    F32R = mybir.dt.float32r
    ACTF = mybir.ActivationFunctionType
    AX = mybir.AxisListType
    OP = mybir.AluOpType

    nc = bacc.Bacc()

    xt = nc.declare_dram_parameter("xt", [IN, XT_COLS], F32, isOutput=False)
    wname = ["wix", "wih", "wox", "woh", "wux", "wuh", "wfx", "wfh"]
    wps = {n: nc.declare_dram_parameter(n, [IN, HID], F32, isOutput=False) for n in wname}
    bT = {g: nc.declare_dram_parameter("bT" + g, [128, 4], F32, isOutput=False) for g in "iouf"}
    out_t = {
        n: nc.declare_dram_parameter(n, [128, 4, 512], F32, isOutput=True)
        for n in ("i3T", "u3T", "o3T", "fc3T")
    }

    def t_view(h):  # DRAM [512, n] -> [128 part, 4 ktile, n] view
        return h[:, :].rearrange("(t p) n -> p t n", p=128)

    from contextlib import ExitStack

    with tile.TileContext(nc) as tc, ExitStack() as ctx:
        consts = ctx.enter_context(tc.tile_pool(name="consts", bufs=1))
        stream = ctx.enter_context(tc.tile_pool(name="stream", bufs=cfg.get("stream", 5)))
        workA = ctx.enter_context(tc.tile_pool(name="workA", bufs=cfg.get("workA", 2)))
        workB = ctx.enter_context(tc.tile_pool(name="workB", bufs=cfg.get("workB", 2)))
        longp = ctx.enter_context(tc.tile_pool(name="longp", bufs=1))
        psum = ctx.enter_context(tc.tile_pool(name="psum", bufs=cfg.get("psum", 8), space="PSUM"))

        # ---------------- constants / weights ----------------
        # Startup critical path: descriptor generation serializes per DMA
        # queue, so the first chunk's x (SP queue), wix (split between Pool
        # and DVE queues) and wux (Pool) are spread to be ready just in time
        # for the i,u,o gate ladder of chunk 0. The h-part weights ride the
        # leaf x stream pool late.
        W = {}
        bTs = {}
        stream_tiles = {}

        def load_w(n, eng=None):
            W[n] = consts.tile([128, 4, HID], F32R, tag="w_" + n, name="w_" + n)
            (eng or nc.sync).dma_start(out=W[n][:, :, :], in_=t_view(wps[n]).bitcast(F32R))

        def load_chunk(c, eng=None):
            t = stream.tile([128, 4, 512], F32R, tag="xt_c", name=f"xt_c{c}")
            (eng or nc.sync).dma_start(
                out=t[:, :, :], in_=t_view(xt)[:, :, c * 512 : (c + 1) * 512].bitcast(F32R)
            )
            stream_tiles[c] = t
            return t

        # DMA transfers serialize in one FIFO ordered by descriptor-gen
        # completion, and SP/ACT queues share the (serial) HWDGE unit; only
        # Pool's SWDGE generates in parallel. So: x chunks and the big
        # weights go on SP in exact consumption order (i0,i1,u0,u1,o0,o1,
        # fx3T,f0 of the paired c0/c1 warmup below), while wix + biases ride
        # the Pool lane whose transfers interleave with SP's early pieces.
        t0 = stream.tile([128, 4, 512], F32R, tag="xt_c", name="xt_c0")
        stream_tiles[0] = t0
        W["wix"] = consts.tile([128, 4, HID], F32R, tag="w_wix", name="w_wix")
        for k in range(4):
            nc.sync.dma_start(out=t0[:, k, :], in_=t_view(xt)[:, k, 0:512].bitcast(F32R))
            nc.gpsimd.dma_start(out=W["wix"][:, k, :], in_=t_view(wps["wix"])[:, k, :].bitcast(F32R))
        for g in "iouf":
            bTs[g] = consts.tile([128, 4], F32, tag="bT" + g, name="bT" + g)
            nc.gpsimd.dma_start(out=bTs[g][:, :], in_=bT[g][:, :])

        # per-k pieces on SP, in exact consumption order, so no big transfer
        # ever sits in the FIFO ahead of an earlier-needed piece
        t1 = stream.tile([128, 4, 512], F32R, tag="xt_c", name="xt_c1")
        stream_tiles[1] = t1
        W["wux"] = consts.tile([128, 4, HID], F32R, tag="w_wux", name="w_wux")
        W["wox"] = consts.tile([128, 4, HID], F32R, tag="w_wox", name="w_wox")
        W["wfx"] = consts.tile([128, 4, HID], F32R, tag="w_wfx", name="w_wfx")
        W["wfh"] = consts.tile([128, 4, HID], F32R, tag="w_wfh", name="w_wfh")
        xt3 = stream.tile([128, 4, 512], F32R, tag="xt_c", name="xt3")
        for k in range(4):
            nc.sync.dma_start(out=t1[:, k, :], in_=t_view(xt)[:, k, 512:1024].bitcast(F32R))
        for k in range(4):
            nc.sync.dma_start(out=W["wux"][:, k, :], in_=t_view(wps["wux"])[:, k, :].bitcast(F32R))
        for k in range(4):
            nc.sync.dma_start(out=xt3[:, k, :], in_=t_view(xt)[:, k, C3_OFF : C3_OFF + 512].bitcast(F32R))
        for k in range(4):
            nc.sync.dma_start(out=W["wfx"][:, k, :], in_=t_view(wps["wfx"])[:, k, :].bitcast(F32R))
        for k in range(4):
            nc.sync.dma_start(out=W["wfh"][:, k, :], in_=t_view(wps["wfh"])[:, k, :].bitcast(F32R))
        for k in range(4):
            nc.sync.dma_start(out=W["wox"][:, k, :], in_=t_view(wps["wox"])[:, k, :].bitcast(F32R))

        # persistent accumulators
        hsum3T = longp.tile([128, 4, 512], F32R, tag="hsum3T")
        fcsum3T = longp.tile([128, 4, 512], F32, tag="fcsum3T")
        fx3T = longp.tile([128, 4, 512], F32, tag="fx3T")

        def gate_T(g, rhs_x, rhs_h, first_k_outer=False):
            """Transposed-layout gate accumulation into 4 single-bank psum
            tiles (one per m) so each bank frees as soon as its m-tile is
            evacuated — PSUM occupancy, not engine time, is the scarce
            resource here (only 8 banks).
            pre[m][:,:] = sum_k WgxT[k,m].T @ rhs_x[k] (+ WghT h-part)"""
            ps = [psum.tile([128, 512], F32, tag="ps", name=f"ps{m}") for m in range(4)]

            if first_k_outer:
                # k-outer so each arriving (x, w) k-tile pair is consumed asap
                for k in range(4):
                    for m in range(4):
                        nc.tensor.matmul(
                            ps[m][:, :], W["w" + g + "x"][:, k, m * 128 : (m + 1) * 128],
                            rhs_x[:, k, :], start=(k == 0), stop=(k == 3),
                        )
                return ps
            for m in range(4):
                ms = slice(m * 128, (m + 1) * 128)
                for k in range(4):
                    nc.tensor.matmul(
                        ps[m][:, :], W["w" + g + "x"][:, k, ms], rhs_x[:, k, :],
                        start=(k == 0), stop=(rhs_h is None and k == 3),
                    )
            if rhs_h is not None:
                # all x-parts first: the h operand (hsum3T) finishes late, so
                # the 16 x matmuls buy PE cover for its arrival
                for m in range(4):
                    ms = slice(m * 128, (m + 1) * 128)
                    for k in range(4):
                        nc.tensor.matmul(
                            ps[m][:, :], W["w" + g + "h"][:, k, ms], rhs_h[:, k, :],
                            start=False, stop=(k == 3),
                        )
            return ps

        def evac(ps, act, bias_g, out_sb):
            for m in range(4):
                b = 0.0 if bias_g is None else bTs[bias_g][:, m : m + 1]
                nc.scalar.activation(out_sb[:, m, :], ps[m][:, :], act, bias=b)

        LOWP = "f32r rounding for downstream matmul"

        def fx3T_compute():
            # fx3T[hid, node] = (x3 @ WfxT) transposed + f bias, kept in the
            # transposed activation layout for the post-matmul broadcast add.
            psx = [psum.tile([128, 512], F32, tag="ps", name=f"ps{m}") for m in range(4)]
            for m in range(4):
                for k in range(4):
                    nc.tensor.matmul(
                        psx[m][:, :], W["wfx"][:, k, m * 128 : (m + 1) * 128],
                        xt3[:, k, :], start=(k == 0), stop=(k == 3),
                    )
            for m in range(4):
                nc.scalar.activation(fx3T[:, m, :], psx[m][:, :], ACTF.Copy)
            # fold the f bias in once (Pool, SBUF in-place) so the per-chunk
            # f sigmoids need no bias and can run as two merged ACT ops
            for m in range(4):
                nc.gpsimd.tensor_scalar_add(fx3T[:, m, :], fx3T[:, m, :],
                                            bTs["f"][:, m : m + 1])

        # ---------------- leaf phase ----------------
        # The f-gate matmuls for chunk c need C(c) (a DVE product of ACT
        # outputs); running them one chunk behind keeps PE from stalling on
        # the ACT/DVE tail of the current chunk.
        def leaf_fpath_mm(c, C_prev):
            # pre_f = Wfh @ C (normal psum group); then DVE adds the
            # broadcast fx3T(+bias) slice reading PSUM directly (per m, so
            # each bank frees in a pipelined wave).
            ps_f = [psum.tile([128, 512], F32, tag="ps", name=f"ps{m}") for m in range(4)]
            for m in range(4):
                ms = slice(m * 128, (m + 1) * 128)
                for k in range(4):
                    nc.tensor.matmul(
                        ps_f[m][:, :], W["wfh"][:, k, ms], C_prev[:, k, :],
                        start=(k == 0), stop=(k == 3),
                    )
            f_sb = workB.tile([128, 4, 512], F32, tag="Ug")
            for m in range(4):
                src = fx3T[:, m, 32 * c : 32 * c + 32]
                nc.vector.tensor_tensor(
                    out=f_sb[:, m, :].rearrange("p (n w) -> p n w", w=16),
                    in0=ps_f[m][:, :].rearrange("p (n w) -> p n w", w=16),
                    in1=src[:, :, None].broadcast_to([128, 32, 16]),
                    op=OP.add,
                )
            return f_sb

        def leaf_fpath_sigma(c, C_prev, f_sb, last=False):
            # sigma is emitted after the current chunk's tanh: its input (the
            # DVE add) lands late, and ACT's in-order queue must not block
            # the next chunk's evacuations behind it
            for h in range(2):
                nc.scalar.activation(f_sb[:, 2 * h : 2 * h + 2, :],
                                     f_sb[:, 2 * h : 2 * h + 2, :], ACTF.Sigmoid)
            fC_sb = workB.tile([128, 4, 512], F32, tag="H")
            # last chunk: fC on DVE (faster than Pool) so fcsum3T completes
            # before the level-3 out-DMAs queue up behind fc3T's transfer
            eng = nc.vector if last else nc.gpsimd
            eng.tensor_mul(fC_sb[:, :, :], f_sb[:, :, :], C_prev[:, :, :].bitcast(F32))
            nc.vector.tensor_reduce(
                fcsum3T[:, :, 32 * c : 32 * c + 32],
                fC_sb[:, :, :].rearrange("p t (g w) -> p t g w", w=16),
                axis=AX.X, op=OP.add,
            )

        def leaf_hpath(c, C_cur, o_cur):
            tC_sb = workA.tile([128, 4, 512], F32, tag="T")
            H_sb = workB.tile([128, 4, 512], F32, tag="H")
            nc.scalar.activation(tC_sb[:, :, :], C_cur[:, :, :].bitcast(F32), ACTF.Tanh)
            nc.vector.tensor_mul(H_sb[:, :, :], o_cur[:, :, :], tC_sb[:, :, :])
            with nc.allow_low_precision(LOWP):
                nc.vector.tensor_reduce(
                    hsum3T[:, :, 32 * c : 32 * c + 32],
                    H_sb[:, :, :].rearrange("p t (g w) -> p t g w", w=16),
                    axis=AX.X, op=OP.add,
                )

        # Paired c0/c1 warmup: six gate waves in a row give the serial DMA
        # FIFO ~22us of PE cover to stream x(c0), wix, x(c1), wux, wox
        # before each is first consumed.
        def gate(g, xt_c, act, bias_g, pool, tag, first_k_outer=False):
            ps = gate_T(g, xt_c, None, first_k_outer=first_k_outer)
            sb = pool.tile([128, 4, 512], F32, tag=tag)
            evac(ps, act, bias_g, sb)
            return sb

        xt_c1 = stream_tiles[1]
        i_sb0 = gate("i", t0, ACTF.Sigmoid, "i", workA, "A", first_k_outer=True)
        i_sb1 = gate("i", xt_c1, ACTF.Sigmoid, "i", workA, "A")
        u_sb0 = gate("u", t0, ACTF.Tanh, "u", workB, "Ug")
        u_sb1 = gate("u", xt_c1, ACTF.Tanh, "u", workB, "Ug")
        C_sb0 = workA.tile([128, 4, 512], F32R, tag="C")
        nc.vector.tensor_mul(C_sb0[:, :, :], i_sb0[:, :, :], u_sb0[:, :, :])
        fx3T_compute()
        f_sb0 = leaf_fpath_mm(0, C_sb0)
        o_sb0 = gate("o", t0, ACTF.Sigmoid, "o", workB, "B")
        o_sb1 = gate("o", xt_c1, ACTF.Sigmoid, "o", workB, "B")
        C_sb1 = workA.tile([128, 4, 512], F32R, tag="C")
        nc.vector.tensor_mul(C_sb1[:, :, :], i_sb1[:, :, :], u_sb1[:, :, :])

        leaf_hpath(0, C_sb0, o_sb0)
        leaf_fpath_sigma(0, C_sb0, f_sb0)
        leaf_hpath(1, C_sb1, o_sb1)

        pipe = (1, C_sb1)
        for c in range(2, N_CHUNK):
            xt_c = load_chunk(c)
            if c >= 12 and c <= 14:
                # late h-part weights, one per iteration: the queue is idle
                # here and they ride spare stream-pool slots until level 3
                n = ("wih", "woh", "wuh")[c - 12]
                W[n] = stream.tile([128, 4, HID], F32R, tag="xt_c", name="w_" + n)
                nc.gpsimd.dma_start(out=W[n][:, :, :], in_=t_view(wps[n]).bitcast(F32R))
            if c == 15:
                # xt3's warmup slot was recycled after fx3T; reload for L3
                xt3_2 = stream.tile([128, 4, 512], F32R, tag="xt_c", name="xt3_2")
                nc.sync.dma_start(
                    out=xt3_2[:, :, :],
                    in_=t_view(xt)[:, :, C3_OFF : C3_OFF + 512].bitcast(F32R),
                )

            i_sb = gate("i", xt_c, ACTF.Sigmoid, "i", workA, "A")
            u_sb = gate("u", xt_c, ACTF.Tanh, "u", workB, "Ug")
            if c == 15:
                # last chunk: C15 -> tanh -> H -> hsum15 is the critical
                # chain into the level-3 h-parts; emit it at queue heads
                C_sb = workA.tile([128, 4, 512], F32R, tag="C")
                nc.vector.tensor_mul(C_sb[:, :, :], i_sb[:, :, :], u_sb[:, :, :])
                f_sb = leaf_fpath_mm(pipe[0], pipe[1])
                o_sb = gate("o", xt_c, ACTF.Sigmoid, "o", workB, "B")
                leaf_hpath(c, C_sb, o_sb)
                leaf_fpath_sigma(pipe[0], pipe[1], f_sb)
            else:
                f_sb = leaf_fpath_mm(pipe[0], pipe[1])
                o_sb = gate("o", xt_c, ACTF.Sigmoid, "o", workB, "B")

                C_sb = workA.tile([128, 4, 512], F32R, tag="C")
                nc.vector.tensor_mul(C_sb[:, :, :], i_sb[:, :, :], u_sb[:, :, :])

                leaf_hpath(c, C_sb, o_sb)
                leaf_fpath_sigma(pipe[0], pipe[1], f_sb)
            pipe = (c, C_sb)

        f_sb = leaf_fpath_mm(pipe[0], pipe[1])
        leaf_fpath_sigma(pipe[0], pipe[1], f_sb, last=True)

        # ---------------- level 3 (512 nodes, transposed) ----------------
        # fcsum3T's DMA is emitted first so its transfer leads the out FIFO.
        nc.sync.dma_start(out=out_t["fc3T"][:, :, :], in_=fcsum3T[:, :, :])

        def l3_gate(g, act, out_name, sb_pool, sb_tag):
            ps = gate_T(g, xt3_2, hsum3T)
            sb = sb_pool.tile([128, 4, 512], F32, tag=sb_tag)
            for m in range(4):
                nc.scalar.activation(sb[:, m, :], ps[m][:, :], act,
                                     bias=bTs[g][:, m : m + 1])
                nc.sync.dma_start(out=out_t[out_name][:, m, :], in_=sb[:, m, :])
            return sb

        l3_gate("i", ACTF.Sigmoid, "i3T", workA, "A")
        l3_gate("u", ACTF.Tanh, "u3T", workB, "Ug")
        l3_gate("o", ACTF.Sigmoid, "o3T", workB, "B")

    nc.finalize()
    return nc


def _np_sigmoid(v):
    return 1.0 / (1.0 + np.exp(-v))


def _host_prep(x, wi_w, wo_w, wu_w, wf_w, wi_b, wo_b, wu_b, wf_b):
    xt_full = np.ascontiguousarray(x.T)  # [512, 69905]

    def wT(w, part):
        return np.ascontiguousarray(w[:, :512].T if part == "x" else w[:, 512:].T)

    common = {
        "wix": wT(wi_w, "x"), "wih": wT(wi_w, "h"),
        "wox": wT(wo_w, "x"), "woh": wT(wo_w, "h"),
        "wux": wT(wu_w, "x"), "wuh": wT(wu_w, "h"),
        "wfx": wT(wf_w, "x"), "wfh": wT(wf_w, "h"),
        "bTi": np.ascontiguousarray(np.asarray(wi_b).reshape(4, 128).T),
        "bTo": np.ascontiguousarray(np.asarray(wo_b).reshape(4, 128).T),
        "bTu": np.ascontiguousarray(np.asarray(wu_b).reshape(4, 128).T),
        "bTf": np.ascontiguousarray(np.asarray(wf_b).reshape(4, 128).T),
    }
    in_maps = []
    for c in range(NC_N):
        xt_c = np.concatenate(
            [
                xt_full[:, OFFS[4] + 8192 * c : OFFS[4] + 8192 * (c + 1)],
                xt_full[:, OFFS[3] + 512 * c : OFFS[3] + 512 * (c + 1)],
            ],
            axis=1,
        )
        in_maps.append({"xt": np.ascontiguousarray(xt_c), **common})
    return in_maps


def _t_to_nodes(a):
    """[128, 4, n] transposed tile -> [n, 512] natural (hid = t*128 + p)."""
    return np.ascontiguousarray(np.transpose(np.asarray(a), (2, 1, 0)).reshape(a.shape[2], 512))


def _host_finish(x, res, wi_w, wi_b, wf_w, wf_b, wo_w, wo_b, wu_w, wu_b):
    """Levels 2..0 in float64 from per-core level-3 gate outputs."""
    f8 = np.float64
    i3 = np.concatenate([_t_to_nodes(res[c]["i3T"]) for c in range(NC_N)]).astype(f8)
    u3 = np.concatenate([_t_to_nodes(res[c]["u3T"]) for c in range(NC_N)]).astype(f8)
    o3 = np.concatenate([_t_to_nodes(res[c]["o3T"]) for c in range(NC_N)]).astype(f8)
    fc3 = np.concatenate([_t_to_nodes(res[c]["fc3T"]) for c in range(NC_N)]).astype(f8)

    C = i3 * u3 + fc3  # [4096, 512]
    H = o3 * np.tanh(C)

    wi = np.asarray(wi_w, f8)
    wo = np.asarray(wo_w, f8)
    wu = np.asarray(wu_w, f8)
    wf = np.asarray(wf_w, f8)
    bi, bo, bu, bf = (np.asarray(b, f8) for b in (wi_b, wo_b, wu_b, wf_b))

    offs = OFFS
    for d in range(2, -1, -1):
        Xd = np.asarray(x[offs[d] : offs[d + 1]], f8)  # [n, 512]
        n = Xd.shape[0]
        Hc = H.reshape(n, BRANCH, HID)
        Cc = C.reshape(n, BRANCH, HID)
        h_sum = Hc.sum(axis=1)
        xh = np.concatenate([Xd, h_sum], axis=1)
        i = _np_sigmoid(xh @ wi.T + bi)
        o = _np_sigmoid(xh @ wo.T + bo)
        u = np.tanh(xh @ wu.T + bu)
        fx = Xd @ wf[:, :IN].T  # [n, 512]
        fc = (C @ wf[:, IN:].T).reshape(n, BRANCH, HID)
        f = _np_sigmoid(fc + fx[:, None, :] + bf)
        C = i * u + (f * Cc).sum(axis=1)
        H = o * np.tanh(C)

    return H[0].astype(np.float32), C[0].astype(np.float32)


def _run(in_maps, trace=False):
    from concourse.bass_utils import run_bass_kernel_spmd

    if "nc" not in _CACHE:
        _CACHE["nc"] = _build_nc()
    return run_bass_kernel_spmd(_CACHE["nc"], in_maps, list(range(NC_N)), trace=trace)


def kernel(x, wi_w, wi_b, wf_w, wf_b, wo_w, wo_b, wu_w, wu_b, _trace=False):
    x = np.asarray(x, np.float32)
    in_maps = _host_prep(x, wi_w, wo_w, wu_w, wf_w, wi_b, wo_b, wu_b, wf_b)
    res = _run(in_maps, trace=_trace)
    _CACHE["last_results"] = res
    H0, C0 = _host_finish(x, res.results, wi_w, wi_b, wf_w, wf_b, wo_w, wo_b, wu_w, wu_b)
    return H0, C0

